# revision 6
# baseline (speedup 1.0000x reference)
"""Trainium2 Bass kernel for nn_DiscreteAutoregressiveFlow (sampling, forward).

Math: `inputs` is an exact one-hot [B, L, V] tensor. For a row holding token v:
  net = W[v] + b                      (exact: one-hot @ W picks a row)
  loc = one_hot(argmax(net[:V]));  scale = one_hot(argmax(net[V:]))
  one_hot_multiply -> one-hot at (scale_tok*v) % V   (zero row if scale_tok==0)
  one_hot_add      -> one-hot at (scale_tok*v + loc_tok) % V
So out[row] = one_hot(cmap[v]) with a host-precomputed 64-entry map
(sentinel >= V encodes the zero row). The straight-through softmax residuals
and FFT noise in the reference are O(1e-7) and vanish in norm relative error.

Device pipeline per 128x(r*64) chunk (pure streaming, memory-bound):
  xt   = DMA-in (sync HWDGE)
  xb   = cast f32->bf16            (scalar ACT; enables DVE 2x add mode)
  prod = xb + cmap                 (DVE TT, bf16 2x)
  m    = reduce_max(prod, inner V) (DVE, 1x) = 1 + cmap[tok]/128, exact
  out  = is_equal(1 + iota/128, m) (DVE, 1x) -> exact 0.0/1.0 f32
  DMA-out (sync HWDGE)
Orchestration: ALL in-DMAs are issued on the sync FIFO BEFORE any out-DMA,
so an out-DMA waiting on compute can never head-of-line-block a later
in-DMA (the failure mode that serialized the interleaved version).
All f32/bf16 values involved are exact (c <= 127 and 2^-7 scaling).
Sharding: pure data parallel over B*L rows, 8 cores, no collectives.
"""

import numpy as np

V = 64
P = 128
N_CORES = 8
B, L = 16, 8192
ROWS = B * L                      # 131072
ROWS_PER_CORE = ROWS // N_CORES   # 16384
SENTINEL = 100.0
EPS = 1.0 / 128.0

# rows-per-partition per chunk. Small first chunks start the DVE pipeline
# early, big middle chunks amortize per-instruction overhead, small last
# chunks shorten the drain tail. sum(R_LIST) * 128 = rows per core.
R_LIST = (4, 8, 24, 32, 32, 20, 8)

_CACHE = {}


def _build_nc(rows_per_core: int, r_list):
    import concourse.bacc as bacc
    import concourse.mybir as mybir
    from concourse.bass import broadcast_tensor_aps
    from concourse.tile import TileContext

    f32 = mybir.dt.float32
    bf16 = mybir.dt.bfloat16
    n_chunks = len(r_list)
    r_max = max(r_list)
    assert rows_per_core == P * sum(r_list)
    # row offset (in r units) of each chunk
    r_off = [0]
    for r in r_list:
        r_off.append(r_off[-1] + r)

    # Bacc (not raw Bass): its compile() runs generate_event_semaphores(),
    # which legalizes multi-wait instructions for TRN2 (1 wait per instr).
    nc = bacc.Bacc("TRN2", target_bir_lowering=False, name="daf_onehot")
    x = nc.dram_tensor("x", [rows_per_core, V], f32, kind="ExternalInput")
    cmap = nc.dram_tensor("cmap", [P, V], f32, kind="ExternalInput")
    iota = nc.dram_tensor("iota", [P, V], f32, kind="ExternalInput")
    y = nc.dram_tensor("y", [rows_per_core, V], f32, kind="ExternalOutput")

    # [p, rtot, v] view with chunks sliced along rtot: chunk ci covers
    # rows [p*rtot + r_off[ci] , ... + r_list[ci]) -- i.e. each partition
    # holds rtot consecutive rows, split among chunks.
    rtot = sum(r_list)
    xv = x.rearrange("(p r) v -> p r v", p=P, r=rtot)
    yv = y.rearrange("(p r) v -> p r v", p=P, r=rtot)

    with TileContext(nc) as tc:
        with (
            tc.tile_pool(name="const", bufs=1) as constp,
            tc.tile_pool(name="io", bufs=1) as iop,
            tc.tile_pool(name="work", bufs=1) as workp,
        ):
            # In-DMAs first in the sync FIFO: chunk 0's data (small) lands
            # immediately; nothing is queued ahead of it.
            xts = []
            for ci, r in enumerate(r_list):
                xt = iop.tile([P, r * V], f32, tag=f"x{ci}")
                x3 = xt[:].rearrange("p (r v) -> p r v", v=V)
                nc.sync.dma_start(x3, xv[:, r_off[ci] : r_off[ci + 1]])
                xts.append(xt)

            # Constants ride the scalar (ACT) HWDGE ring so they never
            # delay the x stream.
            cmap_st = constp.tile([P, V], f32, tag="cmap_st")
            iota_st = constp.tile([P, V], f32, tag="iota_st")
            nc.scalar.dma_start(cmap_st[:], cmap[:])
            nc.scalar.dma_start(iota_st[:], iota[:])
            cmap_1 = cmap_st[:].rearrange("p (o v) -> p o v", o=1)
            iota_1 = iota_st[:].rearrange("p (o v) -> p o v", o=1)

            # Materialized bf16 broadcast tables (step-1 operands for DVE).
            cmap_f = constp.tile([P, r_max * V], bf16, tag="cmap_f")
            cf3 = cmap_f[:].rearrange("p (r v) -> p r v", v=V)
            cm_b, _ = broadcast_tensor_aps(cmap_1, cf3)
            nc.scalar.copy(cf3, cm_b)
            iota_f = constp.tile([P, r_max * V], bf16, tag="iota_f")
            if3 = iota_f[:].rearrange("p (r v) -> p r v", v=V)
            io_b0, _ = broadcast_tensor_aps(iota_1, if3)
            nc.scalar.copy(if3, io_b0)

            outs = []
            for ci, r in enumerate(r_list):
                fd = r * V
                xt = xts[ci]
                xb_d = workp.tile([P, fd], bf16, tag=f"xb{ci}")
                nc.scalar.copy(xb_d[:], xt[:])

                prod = workp.tile([P, fd], bf16, tag=f"prod{ci}")
                p3 = prod[:].rearrange("p (r v) -> p r v", v=V)
                nc.vector.tensor_tensor(
                    prod[:], xb_d[:], cmap_f[:, :fd], op=mybir.AluOpType.add
                )

                c_t = workp.tile([P, r], f32, tag=f"c{ci}")
                nc.vector.tensor_reduce(
                    c_t[:], p3, axis=mybir.AxisListType.X, op=mybir.AluOpType.max
                )

                out_t = iop.tile([P, fd], f32, tag=f"out{ci}")
                o3 = out_t[:].rearrange("p (r v) -> p r v", v=V)
                c3 = c_t[:].rearrange("p (r one) -> p r one", one=1)
                c3_b, _ = broadcast_tensor_aps(c3, o3)
                if3_r = iota_f[:, :fd].rearrange("p (r v) -> p r v", v=V)
                nc.vector.tensor_tensor(o3, if3_r, c3_b, op=mybir.AluOpType.is_equal)
                outs.append(out_t)

            for ci, r in enumerate(r_list):
                o3 = outs[ci][:].rearrange("p (r v) -> p r v", v=V)
                nc.sync.dma_start(yv[:, r_off[ci] : r_off[ci + 1]], o3)

    # Bacc.finalize runs compile(): wait-splitting (generate_event_semaphores),
    # register allocation, nop fusion. run_bass_via_pjrt serializes nc.m as-is,
    # so this must happen here.
    nc.finalize()
    return nc


def _get_nc(rows_per_core=ROWS_PER_CORE, r_list=R_LIST):
    key = (rows_per_core, tuple(r_list))
    if key not in _CACHE:
        _CACHE[key] = _build_nc(rows_per_core, r_list)
    return _CACHE[key]


def _host_cmap(W: np.ndarray, b: np.ndarray) -> np.ndarray:
    """64-entry map token -> output one-hot index (or sentinel for zero row)."""
    net = W.astype(np.float32) + b.astype(np.float32)[None, :]   # [V, 2V]
    loc_tok = np.argmax(net[:, :V], axis=1)                      # [V]
    scale_tok = np.argmax(net[:, V:], axis=1)                    # [V]
    t = (scale_tok * np.arange(V, dtype=np.int64) + loc_tok) % V
    return np.where(scale_tok == 0, SENTINEL, t.astype(np.float64)).astype(
        np.float32
    )


def _host_tables(W: np.ndarray, b: np.ndarray):
    cmap_eps = _host_cmap(W, b) * np.float32(EPS)                  # exact f32
    iota_eps = 1.0 + np.arange(V, dtype=np.float32) * np.float32(EPS)
    cmap_t = np.tile(cmap_eps.astype(np.float32)[None, :], (P, 1))
    iota_t = np.tile(iota_eps.astype(np.float32)[None, :], (P, 1))
    return cmap_t, iota_t


def kernel(inputs: np.ndarray, W: np.ndarray, b: np.ndarray) -> np.ndarray:
    from concourse import bass_utils

    x = np.ascontiguousarray(inputs.astype(np.float32, copy=False).reshape(ROWS, V))
    cmap_t, iota_t = _host_tables(W, b)

    nc = _get_nc()
    in_maps = [
        {
            "x": x[c * ROWS_PER_CORE : (c + 1) * ROWS_PER_CORE],
            "cmap": cmap_t,
            "iota": iota_t,
        }
        for c in range(N_CORES)
    ]
    res = bass_utils.run_bass_kernel_spmd(nc, in_maps, core_ids=list(range(N_CORES)))
    y = np.concatenate([r["y"] for r in res.results], axis=0)
    return y.reshape(inputs.shape).astype(inputs.dtype, copy=False)
